# revision 21
# baseline (speedup 1.0000x reference)
"""RnC (Rank-N-Contrast) loss kernel for Trainium2, 8 NeuronCores.

Math summary (per class of n=256 rows):
  dist[i,j] = ||f_i - f_j||, logits = -dist/2, e = exp(logits)
  L[i,j]    = label distance (class-dependent mix of l1 pairwise dists)
  denom[i,j]= sum_k [L[i,k] >= L[i,j]] e[i,k]   (k=i self-term drops out
              automatically for j!=i because L[i,i] is the row minimum)
  loss      = sum_{i!=j}(log denom - logits) / (n(n-1)) / 6

Key reduction: per row, the multiset {denom[i,j] : j != i} equals the
prefix sums of e taken in descending-L order (positions 0..254), so we
never materialize denom elementwise:
  sum_{j!=i} log denom[i,j] = sum_{q=0..254} log(cumsum_q(e sorted by L desc))

Per-core work: 32 anchors x 6 classes (192 packed rows) vs all 256 class
columns.  Sort = 32 rounds of DVE max8 + match_replace on f32 keys that
carry the column index in the low 8 mantissa bits (rank order of L is
invariant to the per-row monotone transforms we use: +1.0 shift, per-row
additive constants from one-sided zeroing, 256-ulp quantization).
The sorted keys' low bits give the permutation; a gpsimd local_scatter
inverts it and a second local_scatter reorders e (fp16, mean-shifted).
"""

import os
import sys

import numpy as np

for _p in ("/opt/trn_rl_repo", "/root/.axon_site/_ro/trn_rl_repo"):
    if os.path.isdir(_p) and _p not in sys.path:
        sys.path.insert(0, _p)

NUM_CLASSES = 6
GROUP = 256
LAM = 0.8
NCORES = 8
CHUNKS = ((0, 128), (128, 64))  # (row0, nrows) of the packed 192 rows
IND = np.zeros((6, 192), np.float32)
for _c in range(6):
    IND[_c, _c * 32 : (_c + 1) * 32] = 1.0


def _anchor_indices(core: int) -> np.ndarray:
    """Packed-row -> global-row mapping for one core (192 entries)."""
    idx = []
    for c in range(NUM_CLASSES):
        if c != 5:
            idx.extend(range(c * GROUP + core * 32, c * GROUP + core * 32 + 32))
        else:
            # class 5 (mug): balance nonsym (rows 0..127) / sym halves
            idx.extend(range(5 * GROUP + core * 16, 5 * GROUP + core * 16 + 16))
            idx.extend(
                range(5 * GROUP + 128 + core * 16, 5 * GROUP + 128 + core * 16 + 16)
            )
    return np.asarray(idx, dtype=np.int64)


def _build_bass():
    import concourse.bass as bass  # noqa: F401
    import concourse.bacc as bacc
    import concourse.tile as tile
    from concourse import mybir

    f32 = mybir.dt.float32
    f16 = mybir.dt.float16
    u32 = mybir.dt.uint32
    i16 = mybir.dt.int16
    Alu = mybir.AluOpType
    Act = mybir.ActivationFunctionType
    X = mybir.AxisListType.X

    bf16 = mybir.dt.bfloat16
    nc = bacc.Bacc(None)
    ft_d = nc.dram_tensor("ft", [1536, 256], bf16, kind="ExternalInput")
    ftn_d = nc.dram_tensor("ftn", [1536, 32], bf16, kind="ExternalInput")
    gbt_d = nc.dram_tensor("gbt", [6, 2304], f32, kind="ExternalInput")
    gnc_d = nc.dram_tensor("gnc", [192, 9], f32, kind="ExternalInput")
    red5t_d = nc.dram_tensor("red5t", [1, 384], f32, kind="ExternalInput")
    sqj2_d = nc.dram_tensor("sqj2", [2, 1536], f32, kind="ExternalInput")
    sqa2_d = nc.dram_tensor("sqa2", [2, 192], f32, kind="ExternalInput")
    out_d = nc.dram_tensor("out", [192, 3], f32, kind="ExternalOutput")

    with tile.TileContext(nc) as tc:
        with (
            tc.tile_pool(name="persist", bufs=1) as persist,
            tc.tile_pool(name="scratch", bufs=3) as scratch,
            tc.tile_pool(name="psx", bufs=2, space="PSUM") as psx_pool,
        ):
            # ---------------- constants / static tiles ----------------
            gnc_sb = []
            for ci, (r0, nr) in enumerate(CHUNKS):
                t = persist.tile([nr, 9], f32, tag=f"gnc{ci}")
                nc.sync.dma_start(out=t, in_=gnc_d[r0 : r0 + nr, :])
                gnc_sb.append(t)

            sqj2_sb = persist.tile([2, 1536], f32, tag="sqj2")
            nc.sync.dma_start(out=sqj2_sb, in_=sqj2_d[:, :])
            sqa2_sb = persist.tile([2, 192], f32, tag="sqa2")
            nc.sync.dma_start(out=sqa2_sb, in_=sqa2_d[:, :])

            zb = {}
            epsb = {}
            for nrr in (128, 64):
                zb[nrr] = persist.tile([nrr, 1], f32, tag=f"zb{nrr}", name=f"zb{nrr}")
                nc.vector.memset(zb[nrr], 0.0)
                epsb[nrr] = persist.tile(
                    [nrr, 1], f32, tag=f"epsb{nrr}", name=f"epsb{nrr}"
                )
                nc.vector.memset(epsb[nrr], 1e-12)

            iota_i16 = persist.tile([128, 256], i16, tag="iota16")
            nc.gpsimd.iota(iota_i16, pattern=[[1, 256]], channel_multiplier=0)
            iota_u32 = persist.tile([128, 256], u32, tag="iota32")
            nc.gpsimd.iota(iota_u32, pattern=[[1, 256]], channel_multiplier=0)

            # ==========================================================
            # Label-distance L (packed [192,256]) -- sort-critical path
            # ==========================================================
            L_sb = [persist.tile([nr, 256], f32, tag=f"L{ci}", name=f"L{ci}")
                    for ci, (r0, nr) in enumerate(CHUNKS)]
            W_sb = [persist.tile([nr, 256, 9], f32, tag=f"W{ci}", name=f"W{ci}")
                    for ci, (r0, nr) in enumerate(CHUNKS)]

            # mug-correction scratch (chunk1 rows 32:48 = class-5 nonsym
            # anchors; correction applies to columns j<128 only)
            # all mug-correction tiles live at base partition 32 (= class-5
            # nonsym rows within chunk 1) so operand base partitions match
            w5 = persist.tile([48, 128, 6], f32, tag="w5")
            rbc_sb = persist.tile([48, 3, 128], f32, tag="rbc")
            rfull = red5t_d[:, :]
            nc.sync.dma_start(
                out=rbc_sb[32:48, :, :],
                in_=bass.AP(tensor=rfull.tensor, offset=0, ap=[[0, 16], [1, 384]]),
            )

            # row-side gt broadcast via partition-replicating DMA:
            # bc[ci][(c,i), (d,j)] = gbt[c, d*256+j] for the chunk's classes
            bc_sb = []
            for ci, (r0, nr) in enumerate(CHUNKS):
                t = persist.tile([nr, 2304], f32, tag=f"bc{ci}", name=f"bc{ci}")
                gfull = gbt_d[:, :]
                src = bass.AP(
                    tensor=gfull.tensor,
                    offset=(r0 // 32) * 2304,
                    ap=[[2304, nr // 32], [0, 32], [1, 2304]],
                )
                nc.sync.dma_start(out=t[:, :], in_=src)
                bc_sb.append(t)

            for ci, (r0, nr) in enumerate(CHUNKS):
                for d in range(9):
                    bsl = bc_sb[ci][:, d * 256 : (d + 1) * 256]
                    wsl = W_sb[ci][:, :, d : d + 1].rearrange("p n o -> p (n o)")
                    gcol = gnc_sb[ci][:, d : d + 1]
                    nc.scalar.activation(wsl, bsl, Act.Abs, bias=gcol)
                    if ci == 1 and d < 3:
                        # class-5 green |diff| block for the mug correction
                        nc.vector.tensor_scalar(
                            out=w5[32:48, :, d : d + 1].rearrange("p n o -> p (n o)"),
                            in0=bsl[32:48, 0:128],
                            scalar1=gnc_sb[1][32:48, d : d + 1],
                            scalar2=None, op0=Alu.add,
                        )
                nc.vector.tensor_reduce(
                    out=L_sb[ci], in_=W_sb[ci], axis=X, op=Alu.add,
                    apply_absolute_value=True,
                )
                # +1.0: keys become normal floats; rank order unchanged
                nc.vector.tensor_scalar(
                    out=L_sb[ci], in0=L_sb[ci], scalar1=1.0, scalar2=None, op0=Alu.add
                )

            # mug correction: L5[nonsym, j<128] += 0.5*(l1red - l1green)
            for d in range(3):
                nc.vector.tensor_scalar(
                    out=w5[32:48, :, 3 + d : 4 + d].rearrange("p n o -> p (n o)"),
                    in0=rbc_sb[32:48, d, :],
                    scalar1=gnc_sb[1][32:48, 3 + d : 4 + d],
                    scalar2=None, op0=Alu.add,
                )
            a5 = scratch.tile([48, 128], f32, tag="a5")
            b5 = scratch.tile([48, 128], f32, tag="b5")
            nc.vector.tensor_reduce(
                out=a5[32:48, :], in_=w5[32:48, :, 0:3], axis=X, op=Alu.add,
                apply_absolute_value=True,
            )
            nc.vector.tensor_reduce(
                out=b5[32:48, :], in_=w5[32:48, :, 3:6], axis=X, op=Alu.add,
                apply_absolute_value=True,
            )
            nc.vector.tensor_sub(b5[32:48, :], b5[32:48, :], a5[32:48, :])
            Lblk = L_sb[1][32:48, 0:128]
            nc.vector.scalar_tensor_tensor(
                out=Lblk, in0=b5[32:48, :], scalar=0.5, in1=Lblk,
                op0=Alu.mult, op1=Alu.add,
            )

            # ==========================================================
            # Sort: keys = (bits(L) & ~0xFF) | j ; 32x (max8, match_replace)
            # ==========================================================
            sortedK = []
            for ci, (r0, nr) in enumerate(CHUNKS):
                ku = persist.tile([nr, 256], u32, tag=f"ku{ci}")
                nc.vector.tensor_scalar(
                    out=ku, in0=L_sb[ci][:, :].bitcast(u32),
                    scalar1=0xFFFFFF00, scalar2=None, op0=Alu.bitwise_and,
                )
                nc.vector.tensor_tensor(
                    out=ku, in0=ku, in1=iota_u32[0:nr, :], op=Alu.bitwise_or
                )
                sk = persist.tile([nr, 256], f32, tag=f"sk{ci}")
                kf = ku[:, :].bitcast(f32)
                for r in range(32):
                    nc.vector.max(out=sk[:, r * 8 : (r + 1) * 8], in_=kf)
                    nc.vector.match_replace(
                        out=kf,
                        in_to_replace=sk[:, r * 8 : (r + 1) * 8],
                        in_values=kf,
                        imm_value=-3.0e38,
                    )
                sortedK.append(sk)

            # ==========================================================
            # Feature path (PE/ACT heavy, overlaps the DVE sort):
            # psX = -2 f.fT + sq_j + sq_i  (all assembled in PSUM)
            # ==========================================================
            ft_sb, ftn_sb = [], []
            for a in range(12):
                t = persist.tile([128, 256], bf16, tag=f"ft{a}")
                nc.sync.dma_start(out=t, in_=ft_d[a * 128 : (a + 1) * 128, :])
                ft_sb.append(t)
                t2 = persist.tile([128, 32], bf16, tag=f"ftn{a}")
                nc.sync.dma_start(out=t2, in_=ftn_d[a * 128 : (a + 1) * 128, :])
                ftn_sb.append(t2)

            psX = [psx_pool.tile([nr, 256], f32, tag="psx", name=f"psx{ci}")
                   for ci, (r0, nr) in enumerate(CHUNKS)]
            for c in range(6):
                ci = c // 4
                off = c * 32 - CHUNKS[ci][0]
                dst = psX[ci][off : off + 32, :]
                for k in range(2):
                    nc.tensor.matmul(
                        dst, ftn_sb[c * 2 + k], ft_sb[c * 2 + k],
                        start=(k == 0), stop=False, tile_position=(0, off),
                    )
                # += sq_i + sq_j via one K=2 f32 matmul:
                # lhsT = [sq_anchor; 1], rhs = [1; sq_row]
                nc.tensor.matmul(
                    dst, sqa2_sb[:, c * 32 : (c + 1) * 32],
                    sqj2_sb[:, c * 256 : (c + 1) * 256],
                    start=False, stop=True, tile_position=(0, off),
                )

            # q = sqrt(max(dist2,0)+1e-12); e = exp(-q/2 + shift)
            Q_sb, qsum_sb, E_sb = [], [], []
            for ci, (r0, nr) in enumerate(CHUNKS):
                xq = scratch.tile([nr, 256], f32, tag=f"xq{ci}")
                nc.scalar.activation(xq, psX[ci], Act.Relu, bias=zb[nr])
                q = persist.tile([nr, 256], f32, tag=f"q{ci}")
                qsum = persist.tile([nr, 1], f32, tag=f"qsum{ci}")
                nc.scalar.activation(q, xq, Act.Sqrt, bias=epsb[nr], accum_out=qsum)
                # shift = qsum/512 - 1.5: centers e in fp16 range AND keeps
                # the (excluded) self element finite; the -1.5 cancels via a
                # +382.5/row constant folded in on the host
                shift = persist.tile([nr, 1], f32, tag=f"shift{ci}")
                nc.vector.tensor_scalar(
                    out=shift, in0=qsum, scalar1=1.0 / 512.0, scalar2=-1.5,
                    op0=Alu.mult, op1=Alu.add,
                )
                e = persist.tile([nr, 256], f16, tag=f"e{ci}")
                nc.scalar.activation(e, q, Act.Exp, bias=shift, scale=-0.5)
                Q_sb.append(q)
                qsum_sb.append(qsum)
                E_sb.append(e)

            # ==========================================================
            # Permute e by sort order, cumsum, log, reduce, write out
            # ==========================================================
            from concourse import library_config

            nc.gpsimd.load_library(library_config.local_scatter)
            for ci, (r0, nr) in enumerate(CHUNKS):
                idxu = scratch.tile([nr, 256], u32, tag=f"idxu{ci}")
                nc.vector.tensor_scalar(
                    out=idxu, in0=sortedK[ci][:, :].bitcast(u32),
                    scalar1=0xFF, scalar2=None, op0=Alu.bitwise_and,
                )
                idx16 = persist.tile([nr, 256], i16, tag=f"idx16{ci}")
                nc.vector.tensor_copy(out=idx16, in_=idxu)
                rank16 = persist.tile([nr, 256], i16, tag=f"rank16{ci}")
                nc.gpsimd.local_scatter(
                    rank16, iota_i16[0:nr, :], idx16,
                    channels=nr, num_elems=256, num_idxs=256,
                )
                es = persist.tile([nr, 256], f16, tag=f"es{ci}")
                nc.gpsimd.local_scatter(
                    es, E_sb[ci], rank16,
                    channels=nr, num_elems=256, num_idxs=256,
                )
                csum = persist.tile([nr, 256], f32, tag=f"csum{ci}")
                nc.vector.tensor_tensor_scan(
                    out=csum, data0=es, data1=es,
                    initial=0.0, op0=Alu.add, op1=Alu.bypass,
                )
                logsum = persist.tile([nr, 1], f32, tag=f"logsum{ci}")
                nc.scalar.activation(
                    csum[:, 0:255], csum[:, 0:255], Act.Ln, bias=zb[nr],
                    accum_out=logsum,
                )
                # contrib = logsum + qsum/512  (the -0.5*q_ii const is folded
                # in on the host)
                res = persist.tile([nr, 3], f32, tag=f"res{ci}")
                nc.vector.scalar_tensor_tensor(
                    out=res[:, 0:1], in0=qsum_sb[ci], scalar=1.0 / 512.0,
                    in1=logsum, op0=Alu.mult, op1=Alu.add,
                )
                nc.vector.tensor_copy(out=res[:, 1:2], in_=logsum)
                nc.vector.tensor_copy(out=res[:, 2:3], in_=qsum_sb[ci])
                nc.sync.dma_start(out=out_d[r0 : r0 + nr, :], in_=res)

    nc.finalize()
    return nc


_BASS_CACHE = {}


def _get_bass():
    if "nc" not in _BASS_CACHE:
        _BASS_CACHE["nc"] = _build_bass()
    return _BASS_CACHE["nc"]


def _prep_inputs(features, gt_green, gt_red, gt_trans, sym):
    """Build the per-core input maps."""
    import ml_dtypes

    bf16 = ml_dtypes.bfloat16
    f = np.ascontiguousarray(np.asarray(features, dtype=np.float32))
    fb = f.astype(bf16)
    green = np.asarray(gt_green, dtype=np.float32)
    red = np.asarray(gt_red, dtype=np.float32)
    trans = np.asarray(gt_trans, dtype=np.float32)

    # ft: per class block, features transposed (bf16)
    ft = np.ascontiguousarray(
        fb.reshape(6, 256, 256).transpose(0, 2, 1).reshape(1536, 256)
    )
    # squared norms of the bf16-rounded features (keeps the dist diagonal
    # exactly zero against the bf16 Gram)
    sq = (fb.astype(np.float32) ** 2).sum(1).astype(np.float32)  # [1536]
    sqj2 = np.ascontiguousarray(
        np.stack([np.ones(1536, np.float32), sq])
    )  # [2, 1536]

    # row-side broadcast matrix gbt [6, 9*256]: per class scaled
    # [green(3), red(3), trans(3)]; red zeroed for classes 0,1,5
    g_s = np.zeros((6, 256, 3), np.float32)
    r_s = np.zeros((6, 256, 3), np.float32)
    t_s = np.zeros((6, 256, 3), np.float32)
    gg = green.reshape(6, 256, 3)
    rr = red.reshape(6, 256, 3)
    tt = trans.reshape(6, 256, 3)
    for c in range(6):
        if c in (0, 1):
            g_s[c] = LAM * gg[c]
        elif c in (2, 3, 4):
            g_s[c] = 0.5 * LAM * gg[c]
            r_s[c] = 0.5 * LAM * rr[c]
        else:
            g_s[c] = LAM * gg[c]
            # red stays zero on the row side for class 5
        t_s[c] = (1.0 - LAM) * tt[c]
    # layout [6, d*256 + j]
    gbt = np.concatenate([g_s, r_s, t_s], axis=2)  # [6, 256, 9]
    gbt = np.ascontiguousarray(gbt.transpose(0, 2, 1).reshape(6, 9 * 256))

    # class-5 nonsym red row side (columns j<128), scaled by LAM
    red5t = np.ascontiguousarray((LAM * rr[5, :128, :]).T.reshape(1, 384))

    # per-core column scalars gnc [192, 9] = NEGATED scaled gt at anchors
    per_core = []
    for m in range(NCORES):
        rows = _anchor_indices(m)
        cols = np.zeros((192, 9), np.float32)
        gg_a = green[rows]
        rr_a = red[rows]
        tt_a = trans[rows]
        for bi, c in enumerate(range(6)):
            sl = slice(bi * 32, bi * 32 + 32)
            if c in (0, 1):
                cols[sl, 0:3] = LAM * gg_a[sl]
                # red cols zero (unused; row side also zero)
            elif c in (2, 3, 4):
                cols[sl, 0:3] = 0.5 * LAM * gg_a[sl]
                cols[sl, 3:6] = 0.5 * LAM * rr_a[sl]
            else:
                cols[sl, 0:3] = LAM * gg_a[sl]
                # class-5 red cols: only enter |0 - red_i| (a per-row
                # constant, rank-safe) and the mug-correction block
                cols[sl, 3:6] = LAM * rr_a[sl]
            cols[sl, 6:9] = (1.0 - LAM) * tt_a[sl]
        gnc = np.ascontiguousarray(-cols)

        # ftn = -2 * fT[:, anchors] per class block (bf16; -2x is exact)
        ftn = np.zeros((1536, 32), bf16)
        fr = fb.reshape(6, 256, 256)
        for c in range(6):
            local = rows[c * 32 : (c + 1) * 32] - c * 256
            ftn[c * 256 : (c + 1) * 256, :] = (
                -2.0 * fr[c][local, :].astype(np.float32)
            ).astype(bf16).T
        sqa2 = np.ascontiguousarray(
            np.stack([sq[rows], np.ones(192, np.float32)])
        )  # [2, 192]
        per_core.append(
            {
                "ft": ft,
                "ftn": np.ascontiguousarray(ftn),
                "gbt": gbt,
                "gnc": gnc,
                "red5t": red5t,
                "sqj2": sqj2,
                "sqa2": sqa2,
            }
        )
    return per_core


def kernel(features, labels, gt_green, gt_red, gt_trans, sym):
    from concourse.bass_utils import run_bass_kernel_spmd

    nc = _get_bass()
    in_maps = _prep_inputs(features, gt_green, gt_red, gt_trans, sym)
    res = run_bass_kernel_spmd(
        nc, in_maps, core_ids=list(range(NCORES)),
        trace=bool(os.environ.get("BASS_TRACE")),
    )
    _BASS_CACHE["last_results"] = res
    total = 0.0
    for r in res.results:
        total += float(np.asarray(r["out"][:, 0], dtype=np.float64).sum())
    # fold in the constant -0.5*q_ii (= -0.5e-6) per row, then normalize
    total += 1536 * (255.0 * 1.5 - 0.5e-6)
    loss = total / (GROUP * (GROUP - 1)) / NUM_CLASSES
    return np.float32(loss)


# revision 22
# speedup vs baseline: 1.2832x; 1.2832x over previous
"""RnC (Rank-N-Contrast) loss kernel for Trainium2, 8 NeuronCores.

Math summary (per class of n=256 rows):
  dist[i,j] = ||f_i - f_j||, logits = -dist/2, e = exp(logits)
  L[i,j]    = label distance (class-dependent mix of l1 pairwise dists)
  denom[i,j]= sum_k [L[i,k] >= L[i,j]] e[i,k]   (k=i self-term drops out
              automatically for j!=i because L[i,i] is the row minimum)
  loss      = sum_{i!=j}(log denom - logits) / (n(n-1)) / 6

Key reduction: per row, the multiset {denom[i,j] : j != i} equals the
prefix sums of e taken in descending-L order (positions 0..254), so we
never materialize denom elementwise:
  sum_{j!=i} log denom[i,j] = sum_{q=0..254} log(cumsum_q(e sorted by L desc))

Per-core work: 32 anchors x 6 classes (192 packed rows) vs all 256 class
columns.  Sort = 32 rounds of DVE max8 + match_replace on f32 keys that
carry the column index in the low 8 mantissa bits (rank order of L is
invariant to the per-row monotone transforms we use: +1.0 shift, per-row
additive constants from one-sided zeroing, 256-ulp quantization).
The sorted keys' low bits give the permutation; a gpsimd local_scatter
inverts it and a second local_scatter reorders e (fp16, mean-shifted).
"""

import os
import sys

import numpy as np

for _p in ("/opt/trn_rl_repo", "/root/.axon_site/_ro/trn_rl_repo"):
    if os.path.isdir(_p) and _p not in sys.path:
        sys.path.insert(0, _p)

NUM_CLASSES = 6
GROUP = 256
LAM = 0.8
NCORES = 8
CHUNKS = ((0, 128), (128, 64))  # (row0, nrows) of the packed 192 rows
IND = np.zeros((6, 192), np.float32)
for _c in range(6):
    IND[_c, _c * 32 : (_c + 1) * 32] = 1.0


def _anchor_indices(core: int) -> np.ndarray:
    """Packed-row -> global-row mapping for one core (192 entries)."""
    idx = []
    for c in range(NUM_CLASSES):
        if c != 5:
            idx.extend(range(c * GROUP + core * 32, c * GROUP + core * 32 + 32))
        else:
            # class 5 (mug): balance nonsym (rows 0..127) / sym halves
            idx.extend(range(5 * GROUP + core * 16, 5 * GROUP + core * 16 + 16))
            idx.extend(
                range(5 * GROUP + 128 + core * 16, 5 * GROUP + 128 + core * 16 + 16)
            )
    return np.asarray(idx, dtype=np.int64)


def _build_bass():
    import concourse.bass as bass  # noqa: F401
    import concourse.bacc as bacc
    import concourse.tile as tile
    from concourse import mybir

    f32 = mybir.dt.float32
    f16 = mybir.dt.float16
    u32 = mybir.dt.uint32
    i16 = mybir.dt.int16
    Alu = mybir.AluOpType
    Act = mybir.ActivationFunctionType
    X = mybir.AxisListType.X

    bf16 = mybir.dt.bfloat16
    nc = bacc.Bacc(None)
    ft_d = nc.dram_tensor("ft", [1536, 256], bf16, kind="ExternalInput")
    ftn_d = nc.dram_tensor("ftn", [1536, 32], bf16, kind="ExternalInput")
    gbt_d = nc.dram_tensor("gbt", [6, 2304], bf16, kind="ExternalInput")
    ind_d = nc.dram_tensor("ind", [6, 192], bf16, kind="ExternalInput")
    gnc_d = nc.dram_tensor("gnc", [192, 9], f32, kind="ExternalInput")
    red5t_d = nc.dram_tensor("red5t", [1, 384], f32, kind="ExternalInput")
    sqj2_d = nc.dram_tensor("sqj2", [2, 1536], f32, kind="ExternalInput")
    sqa2_d = nc.dram_tensor("sqa2", [2, 192], f32, kind="ExternalInput")
    out_d = nc.dram_tensor("out", [192, 3], f32, kind="ExternalOutput")

    with tile.TileContext(nc) as tc:
        with (
            tc.tile_pool(name="persist", bufs=1) as persist,
            tc.tile_pool(name="scratch", bufs=3) as scratch,
            tc.tile_pool(name="psx", bufs=2, space="PSUM") as psx_pool,
            tc.tile_pool(name="psb", bufs=3, space="PSUM") as psb_pool,
        ):
            # ---------------- constants / static tiles ----------------
            gnc_sb = []
            for ci, (r0, nr) in enumerate(CHUNKS):
                t = persist.tile([nr, 9], f32, tag=f"gnc{ci}")
                nc.sync.dma_start(out=t, in_=gnc_d[r0 : r0 + nr, :])
                gnc_sb.append(t)

            gbt_sb = persist.tile([6, 2304], bf16, tag="gbt")
            nc.sync.dma_start(out=gbt_sb, in_=gbt_d[:, :])
            ind_sb = persist.tile([6, 192], bf16, tag="ind")
            nc.sync.dma_start(out=ind_sb, in_=ind_d[:, :])
            sqj2_sb = persist.tile([2, 1536], f32, tag="sqj2")
            nc.sync.dma_start(out=sqj2_sb, in_=sqj2_d[:, :])
            sqa2_sb = persist.tile([2, 192], f32, tag="sqa2")
            nc.sync.dma_start(out=sqa2_sb, in_=sqa2_d[:, :])

            zb = {}
            epsb = {}
            for nrr in (128, 64):
                zb[nrr] = persist.tile([nrr, 1], f32, tag=f"zb{nrr}", name=f"zb{nrr}")
                nc.vector.memset(zb[nrr], 0.0)
                epsb[nrr] = persist.tile(
                    [nrr, 1], f32, tag=f"epsb{nrr}", name=f"epsb{nrr}"
                )
                nc.vector.memset(epsb[nrr], 1e-12)

            iota_i16 = persist.tile([128, 256], i16, tag="iota16")
            nc.gpsimd.iota(iota_i16, pattern=[[1, 256]], channel_multiplier=0)
            iota_u32 = persist.tile([128, 256], u32, tag="iota32")
            nc.gpsimd.iota(iota_u32, pattern=[[1, 256]], channel_multiplier=0)

            # ==========================================================
            # Label-distance L (packed [192,256]) -- sort-critical path
            # ==========================================================
            L_sb = [persist.tile([nr, 256], f32, tag=f"L{ci}", name=f"L{ci}")
                    for ci, (r0, nr) in enumerate(CHUNKS)]
            W_sb = [persist.tile([nr, 256, 9], f32, tag=f"W{ci}", name=f"W{ci}")
                    for ci, (r0, nr) in enumerate(CHUNKS)]

            # mug-correction scratch (chunk1 rows 32:48 = class-5 nonsym
            # anchors; correction applies to columns j<128 only)
            # all mug-correction tiles live at base partition 32 (= class-5
            # nonsym rows within chunk 1) so operand base partitions match
            w5 = persist.tile([48, 128, 6], f32, tag="w5")
            rbc_sb = persist.tile([48, 3, 128], f32, tag="rbc")
            rfull = red5t_d[:, :]
            nc.sync.dma_start(
                out=rbc_sb[32:48, :, :],
                in_=bass.AP(tensor=rfull.tensor, offset=0, ap=[[0, 16], [1, 384]]),
            )

            # row-side gt broadcast via bf16 PE matmul (indicator outer
            # product): psb[(c,i), j] = gbt[c, d*256+j]
            for ci, (r0, nr) in enumerate(CHUNKS):
                for d in range(9):
                    psb = psb_pool.tile([nr, 256], f32, tag="psb")
                    nc.tensor.matmul(
                        psb, ind_sb[:, r0 : r0 + nr],
                        gbt_sb[:, d * 256 : (d + 1) * 256],
                        start=True, stop=True,
                    )
                    wsl = W_sb[ci][:, :, d : d + 1].rearrange("p n o -> p (n o)")
                    gcol = gnc_sb[ci][:, d : d + 1]
                    nc.scalar.activation(wsl, psb, Act.Abs, bias=gcol)
                    if ci == 1 and d < 3:
                        # class-5 green |diff| block for the mug correction
                        nc.vector.tensor_scalar(
                            out=w5[32:48, :, d : d + 1].rearrange("p n o -> p (n o)"),
                            in0=psb[32:48, 0:128],
                            scalar1=gnc_sb[1][32:48, d : d + 1],
                            scalar2=None, op0=Alu.add,
                        )
                nc.vector.tensor_reduce(
                    out=L_sb[ci], in_=W_sb[ci], axis=X, op=Alu.add,
                    apply_absolute_value=True,
                )
                # +1.0: keys become normal floats; rank order unchanged
                nc.vector.tensor_scalar(
                    out=L_sb[ci], in0=L_sb[ci], scalar1=1.0, scalar2=None, op0=Alu.add
                )

            # mug correction: L5[nonsym, j<128] += 0.5*(l1red - l1green)
            for d in range(3):
                nc.vector.tensor_scalar(
                    out=w5[32:48, :, 3 + d : 4 + d].rearrange("p n o -> p (n o)"),
                    in0=rbc_sb[32:48, d, :],
                    scalar1=gnc_sb[1][32:48, 3 + d : 4 + d],
                    scalar2=None, op0=Alu.add,
                )
            a5 = scratch.tile([48, 128], f32, tag="a5")
            b5 = scratch.tile([48, 128], f32, tag="b5")
            nc.vector.tensor_reduce(
                out=a5[32:48, :], in_=w5[32:48, :, 0:3], axis=X, op=Alu.add,
                apply_absolute_value=True,
            )
            nc.vector.tensor_reduce(
                out=b5[32:48, :], in_=w5[32:48, :, 3:6], axis=X, op=Alu.add,
                apply_absolute_value=True,
            )
            nc.vector.tensor_sub(b5[32:48, :], b5[32:48, :], a5[32:48, :])
            Lblk = L_sb[1][32:48, 0:128]
            nc.vector.scalar_tensor_tensor(
                out=Lblk, in0=b5[32:48, :], scalar=0.5, in1=Lblk,
                op0=Alu.mult, op1=Alu.add,
            )

            # ==========================================================
            # Sort: keys = (bits(L) & ~0xFF) | j ; 32x (max8, match_replace)
            # ==========================================================
            sortedK = []
            for ci, (r0, nr) in enumerate(CHUNKS):
                ku = persist.tile([nr, 256], u32, tag=f"ku{ci}")
                nc.vector.tensor_scalar(
                    out=ku, in0=L_sb[ci][:, :].bitcast(u32),
                    scalar1=0xFFFFFF00, scalar2=None, op0=Alu.bitwise_and,
                )
                nc.vector.tensor_tensor(
                    out=ku, in0=ku, in1=iota_u32[0:nr, :], op=Alu.bitwise_or
                )
                sk = persist.tile([nr, 256], f32, tag=f"sk{ci}")
                kf = ku[:, :].bitcast(f32)
                for r in range(32):
                    nc.vector.max(out=sk[:, r * 8 : (r + 1) * 8], in_=kf)
                    nc.vector.match_replace(
                        out=kf,
                        in_to_replace=sk[:, r * 8 : (r + 1) * 8],
                        in_values=kf,
                        imm_value=-3.0e38,
                    )
                sortedK.append(sk)

            # ==========================================================
            # Feature path (PE/ACT heavy, overlaps the DVE sort):
            # psX = -2 f.fT + sq_j + sq_i  (all assembled in PSUM)
            # ==========================================================
            ft_sb, ftn_sb = [], []
            for a in range(12):
                t = persist.tile([128, 256], bf16, tag=f"ft{a}")
                nc.sync.dma_start(out=t, in_=ft_d[a * 128 : (a + 1) * 128, :])
                ft_sb.append(t)
                t2 = persist.tile([128, 32], bf16, tag=f"ftn{a}")
                nc.sync.dma_start(out=t2, in_=ftn_d[a * 128 : (a + 1) * 128, :])
                ftn_sb.append(t2)

            psX = [psx_pool.tile([nr, 256], f32, tag="psx", name=f"psx{ci}")
                   for ci, (r0, nr) in enumerate(CHUNKS)]
            for c in range(6):
                ci = c // 4
                off = c * 32 - CHUNKS[ci][0]
                dst = psX[ci][off : off + 32, :]
                for k in range(2):
                    nc.tensor.matmul(
                        dst, ftn_sb[c * 2 + k], ft_sb[c * 2 + k],
                        start=(k == 0), stop=False, tile_position=(0, off),
                    )
                # += sq_i + sq_j via one K=2 f32 matmul:
                # lhsT = [sq_anchor; 1], rhs = [1; sq_row]
                nc.tensor.matmul(
                    dst, sqa2_sb[:, c * 32 : (c + 1) * 32],
                    sqj2_sb[:, c * 256 : (c + 1) * 256],
                    start=False, stop=True, tile_position=(0, off),
                )

            # q = sqrt(max(dist2,0)+1e-12); e = exp(-q/2 + shift)
            Q_sb, qsum_sb, E_sb = [], [], []
            for ci, (r0, nr) in enumerate(CHUNKS):
                xq = scratch.tile([nr, 256], f32, tag=f"xq{ci}")
                nc.scalar.activation(xq, psX[ci], Act.Relu, bias=zb[nr])
                q = persist.tile([nr, 256], f32, tag=f"q{ci}")
                qsum = persist.tile([nr, 1], f32, tag=f"qsum{ci}")
                nc.scalar.activation(q, xq, Act.Sqrt, bias=epsb[nr], accum_out=qsum)
                # shift = qsum/512 - 1.5: centers e in fp16 range AND keeps
                # the (excluded) self element finite; the -1.5 cancels via a
                # +382.5/row constant folded in on the host
                shift = persist.tile([nr, 1], f32, tag=f"shift{ci}")
                nc.vector.tensor_scalar(
                    out=shift, in0=qsum, scalar1=1.0 / 512.0, scalar2=-1.5,
                    op0=Alu.mult, op1=Alu.add,
                )
                e = persist.tile([nr, 256], f16, tag=f"e{ci}")
                nc.scalar.activation(e, q, Act.Exp, bias=shift, scale=-0.5)
                Q_sb.append(q)
                qsum_sb.append(qsum)
                E_sb.append(e)

            # ==========================================================
            # Permute e by sort order, cumsum, log, reduce, write out
            # ==========================================================
            from concourse import library_config

            nc.gpsimd.load_library(library_config.local_scatter)
            for ci, (r0, nr) in enumerate(CHUNKS):
                idxu = scratch.tile([nr, 256], u32, tag=f"idxu{ci}")
                nc.vector.tensor_scalar(
                    out=idxu, in0=sortedK[ci][:, :].bitcast(u32),
                    scalar1=0xFF, scalar2=None, op0=Alu.bitwise_and,
                )
                idx16 = persist.tile([nr, 256], i16, tag=f"idx16{ci}")
                nc.vector.tensor_copy(out=idx16, in_=idxu)
                rank16 = persist.tile([nr, 256], i16, tag=f"rank16{ci}")
                nc.gpsimd.local_scatter(
                    rank16, iota_i16[0:nr, :], idx16,
                    channels=nr, num_elems=256, num_idxs=256,
                )
                es = persist.tile([nr, 256], f16, tag=f"es{ci}")
                nc.gpsimd.local_scatter(
                    es, E_sb[ci], rank16,
                    channels=nr, num_elems=256, num_idxs=256,
                )
                csum = persist.tile([nr, 256], f32, tag=f"csum{ci}")
                nc.vector.tensor_tensor_scan(
                    out=csum, data0=es, data1=es,
                    initial=0.0, op0=Alu.add, op1=Alu.bypass,
                )
                logsum = persist.tile([nr, 1], f32, tag=f"logsum{ci}")
                nc.scalar.activation(
                    csum[:, 0:255], csum[:, 0:255], Act.Ln, bias=zb[nr],
                    accum_out=logsum,
                )
                # contrib = logsum + qsum/512  (the -0.5*q_ii const is folded
                # in on the host)
                res = persist.tile([nr, 3], f32, tag=f"res{ci}")
                nc.vector.scalar_tensor_tensor(
                    out=res[:, 0:1], in0=qsum_sb[ci], scalar=1.0 / 512.0,
                    in1=logsum, op0=Alu.mult, op1=Alu.add,
                )
                nc.vector.tensor_copy(out=res[:, 1:2], in_=logsum)
                nc.vector.tensor_copy(out=res[:, 2:3], in_=qsum_sb[ci])
                nc.sync.dma_start(out=out_d[r0 : r0 + nr, :], in_=res)

    nc.finalize()
    return nc


_BASS_CACHE = {}


def _get_bass():
    if "nc" not in _BASS_CACHE:
        _BASS_CACHE["nc"] = _build_bass()
    return _BASS_CACHE["nc"]


def _prep_inputs(features, gt_green, gt_red, gt_trans, sym):
    """Build the per-core input maps."""
    import ml_dtypes

    bf16 = ml_dtypes.bfloat16
    f = np.ascontiguousarray(np.asarray(features, dtype=np.float32))
    fb = f.astype(bf16)
    green = np.asarray(gt_green, dtype=np.float32)
    red = np.asarray(gt_red, dtype=np.float32)
    trans = np.asarray(gt_trans, dtype=np.float32)

    # ft: per class block, features transposed (bf16)
    ft = np.ascontiguousarray(
        fb.reshape(6, 256, 256).transpose(0, 2, 1).reshape(1536, 256)
    )
    # squared norms of the bf16-rounded features (keeps the dist diagonal
    # exactly zero against the bf16 Gram)
    sq = (fb.astype(np.float32) ** 2).sum(1).astype(np.float32)  # [1536]
    sqj2 = np.ascontiguousarray(
        np.stack([np.ones(1536, np.float32), sq])
    )  # [2, 1536]

    # row-side broadcast matrix gbt [6, 9*256]: per class scaled
    # [green(3), red(3), trans(3)]; red zeroed for classes 0,1,5
    g_s = np.zeros((6, 256, 3), np.float32)
    r_s = np.zeros((6, 256, 3), np.float32)
    t_s = np.zeros((6, 256, 3), np.float32)
    gg = green.reshape(6, 256, 3)
    rr = red.reshape(6, 256, 3)
    tt = trans.reshape(6, 256, 3)
    for c in range(6):
        if c in (0, 1):
            g_s[c] = LAM * gg[c]
        elif c in (2, 3, 4):
            g_s[c] = 0.5 * LAM * gg[c]
            r_s[c] = 0.5 * LAM * rr[c]
        else:
            g_s[c] = LAM * gg[c]
            # red stays zero on the row side for class 5
        t_s[c] = (1.0 - LAM) * tt[c]
    # layout [6, d*256 + j]
    gbt = np.concatenate([g_s, r_s, t_s], axis=2)  # [6, 256, 9]
    gbt = np.ascontiguousarray(
        gbt.transpose(0, 2, 1).reshape(6, 9 * 256).astype(bf16)
    )

    # class-5 nonsym red row side (columns j<128), scaled by LAM
    red5t = np.ascontiguousarray((LAM * rr[5, :128, :]).T.reshape(1, 384))

    # per-core column scalars gnc [192, 9] = NEGATED scaled gt at anchors
    per_core = []
    for m in range(NCORES):
        rows = _anchor_indices(m)
        cols = np.zeros((192, 9), np.float32)
        gg_a = green[rows]
        rr_a = red[rows]
        tt_a = trans[rows]
        for bi, c in enumerate(range(6)):
            sl = slice(bi * 32, bi * 32 + 32)
            if c in (0, 1):
                cols[sl, 0:3] = LAM * gg_a[sl]
                # red cols zero (unused; row side also zero)
            elif c in (2, 3, 4):
                cols[sl, 0:3] = 0.5 * LAM * gg_a[sl]
                cols[sl, 3:6] = 0.5 * LAM * rr_a[sl]
            else:
                cols[sl, 0:3] = LAM * gg_a[sl]
                # class-5 red cols: only enter |0 - red_i| (a per-row
                # constant, rank-safe) and the mug-correction block
                cols[sl, 3:6] = LAM * rr_a[sl]
            cols[sl, 6:9] = (1.0 - LAM) * tt_a[sl]
        gnc = np.ascontiguousarray(-cols)

        # ftn = -2 * fT[:, anchors] per class block (bf16; -2x is exact)
        ftn = np.zeros((1536, 32), bf16)
        fr = fb.reshape(6, 256, 256)
        for c in range(6):
            local = rows[c * 32 : (c + 1) * 32] - c * 256
            ftn[c * 256 : (c + 1) * 256, :] = (
                -2.0 * fr[c][local, :].astype(np.float32)
            ).astype(bf16).T
        sqa2 = np.ascontiguousarray(
            np.stack([sq[rows], np.ones(192, np.float32)])
        )  # [2, 192]
        per_core.append(
            {
                "ft": ft,
                "ftn": np.ascontiguousarray(ftn),
                "gbt": gbt,
                "gnc": gnc,
                "red5t": red5t,
                "ind": IND.astype(bf16),
                "sqj2": sqj2,
                "sqa2": sqa2,
            }
        )
    return per_core


def kernel(features, labels, gt_green, gt_red, gt_trans, sym):
    from concourse.bass_utils import run_bass_kernel_spmd

    nc = _get_bass()
    in_maps = _prep_inputs(features, gt_green, gt_red, gt_trans, sym)
    res = run_bass_kernel_spmd(
        nc, in_maps, core_ids=list(range(NCORES)),
        trace=bool(os.environ.get("BASS_TRACE")),
    )
    _BASS_CACHE["last_results"] = res
    total = 0.0
    for r in res.results:
        total += float(np.asarray(r["out"][:, 0], dtype=np.float64).sum())
    # fold in the constant -0.5*q_ii (= -0.5e-6) per row, then normalize
    total += 1536 * (255.0 * 1.5 - 0.5e-6)
    loss = total / (GROUP * (GROUP - 1)) / NUM_CLASSES
    return np.float32(loss)
